# revision 13
# baseline (speedup 1.0000x reference)
"""Trainium2 Bass kernel for causal multi-head attention with RoPE.

nn_CausalAttention: x [2, 2048, 2048], Wq/Wk/Wv [2048, 2048] (y = x @ W.T),
16 heads of dim 128, RoPE, causal fp32 softmax.

Sharding (tensor-parallel heads, per the problem hint): each of the 8
NeuronCores owns 2 heads (a 256-wide slice of the QKV output dim) for both
batch elements. Each core runs the full pipeline for its heads; the full
output is assembled on host by concatenating per-core feature slices (no
collectives needed).

v3 (bf16 operand pipeline):
  All matmul operands are bf16 (PSUM accumulation stays fp32), which turns
  on fast-weight-load, halves HBM traffic, and removes the fp32r
  small-moving-dim penalty so diagonal attention tiles use exact causal
  widths.  Phase A computes q^T/k^T in [head_dim x seq] layout with RoPE
  fused into the PSUM eviction (quadrant-16 layout + stream_shuffle), and v
  directly in [seq x head_dim] layout by using x^T chunks as the stationary
  operand (no PE transposes at all).  Phase B runs causal attention in
  transposed-score layout with exp evaluated two key-tiles per ACTIVATE
  (PSUM quads) to amortize scalar-engine instruction overhead, softmax
  denominator via an all-ones matmul, fast approximate reciprocal fused
  into the output eviction, query tiles processed in descending order so
  the kernel tail drains the smallest tile.
"""

import math

import numpy as np
import ml_dtypes

import concourse.bacc as bacc
import concourse.bass as bass
import concourse.mybir as mybir
import concourse.tile as tile
from concourse import bass_utils

F32 = mybir.dt.float32
BF16 = mybir.dt.bfloat16
AF = mybir.ActivationFunctionType
NPBF16 = ml_dtypes.bfloat16

S = 2048
M = 2048
NCORES = 8

D = 128          # head dim
NH = 2           # heads per core
NB = 2           # batches
SLAB = 512       # phase-A sequence slab
QT = 512         # phase-B query tile


def _rope_perm(n):
    """Row permutation for the quadrant-16 RoPE layout.

    New row p (within a 128-row head block): quadrant qd = p//32, r = p%32.
    r < 16  -> even element of pair i = 16*qd + r      (old row 2i)
    r >= 16 -> odd  element of pair i = 16*qd + (r-16) (old row 2i+1)
    Pair elements are 16 partitions apart inside one 32-partition quadrant,
    so the RoPE combine is a stream_shuffle with a 16-rotation mask.
    """
    perm = []
    for hb in range(n // D):
        base = hb * D
        for qd in range(4):
            perm += [base + 2 * (16 * qd + r) for r in range(16)]
            perm += [base + 2 * (16 * qd + r) + 1 for r in range(16)]
    return np.array(perm)


SWAP16 = [(i + 16) % 32 for i in range(32)]

_PREP_CACHE = {}


def _shared_prep(x):
    key = id(x)
    if _PREP_CACHE.get("key") != key:
        _PREP_CACHE.clear()
        _PREP_CACHE["key"] = key
        _PREP_CACHE["xT0"] = np.ascontiguousarray(x[0].T).astype(NPBF16)
        _PREP_CACHE["xT1"] = np.ascontiguousarray(x[1].T).astype(NPBF16)
    return _PREP_CACHE["xT0"], _PREP_CACHE["xT1"]


def prep_core_inputs(x, Wq, Wk, Wv, core, S, M):
    """Host-side shard prep for one core. x [2,S,M], W* [M', M] where
    rows [core*256, core*256+256) of W* are this core's heads."""
    nsl = slice(core * NH * D, (core + 1) * NH * D)
    perm = _rope_perm(NH * D)
    wq = Wq[nsl][perm]
    wk = Wk[nsl][perm]
    wv = Wv[nsl]

    theta = np.exp(
        -np.float32(np.log(10000.0))
        * (np.arange(0, D, 2, dtype=np.float32) / np.float32(D))
    ).astype(np.float32)
    pos = np.arange(S, dtype=np.float32)
    freqs = theta[:, None] * pos[None, :]  # [64, S], row i = theta_i * s
    cos_t, sin_t = np.cos(freqs), np.sin(freqs)
    # quadrant-16 layout: partition p -> pair i(p) = 16*(p//32) + (p%16)
    p = np.arange(128)
    i_of_p = 16 * (p // 32) + (p % 16)
    is_odd = (p % 32) >= 16
    packC = cos_t[i_of_p].astype(np.float32)                    # [128, S]
    packS = np.where(
        is_odd[:, None], -sin_t[i_of_p], sin_t[i_of_p]
    ).astype(np.float32)

    kk, qq = np.meshgrid(np.arange(128), np.arange(128), indexing="ij")
    tri = (kk <= qq).astype(NPBF16)

    xT0, xT1 = _shared_prep(x)
    return {
        "xT0": xT0,
        "xT1": xT1,
        "wqT": np.ascontiguousarray(wq.T).astype(NPBF16),
        "wkT": np.ascontiguousarray(wk.T).astype(NPBF16),
        "wvT": np.ascontiguousarray(wv.T).astype(NPBF16),
        "packC": packC,
        "packS": packS,
        "tri": tri,
        "ones": np.ones((128, 128), dtype=NPBF16),
    }


def build_attention(tc: tile.TileContext, io: dict, S: int, M: int):
    nc = tc.nc
    xT = [io["xT0"], io["xT1"]]
    outT = io["outT"]

    with (
        tc.tile_pool(name="wpool", bufs=1) as wpool,
        tc.tile_pool(name="constpool", bufs=1) as constpool,
        tc.tile_pool(name="xp", bufs=2) as xpool,
        tc.tile_pool(name="rope", bufs=1) as ropetmp,
    ):
        MC = M // 128
        w_sb = {
            name: wpool.tile([128, MC, NH * D], BF16, tag=name, name=name)
            for name in ("wqT", "wkT", "wvT")
        }
        tri_sb = constpool.tile([128, 128], BF16)
        ones_sb = constpool.tile([128, 128], BF16)
        packC_sb = constpool.tile([128, S], F32)
        packS_sb = constpool.tile([128, S], F32)
        # (DMAs for tri/ones are issued inside phase B, packC/S inside
        #  phase A after the first slab, so they don't delay the critical
        #  first x/w loads)

        for b in range(NB):
            with tc.tile_pool(name=f"qkv{b}", bufs=1) as qkvp:
                qT_sb = qkvp.tile([128, NH, S], BF16, name="qT_sb")
                kT_sb = qkvp.tile([128, NH, S], BF16, name="kT_sb")
                v_sb = qkvp.tile([128, S // 128, NH * D], BF16, name="v_sb")

                phase_a(tc, io, b, xT, w_sb, qT_sb, kT_sb, v_sb,
                        packC_sb, packS_sb, S, M, xpool, ropetmp,
                        load_w=(b == 0))
                phase_b(tc, io, b, outT, ones_sb, tri_sb, qT_sb, kT_sb,
                        v_sb, S)


def phase_a(tc, io, b, xT, w_sb, qT_sb, kT_sb, v_sb, packC_sb, packS_sb,
            S, M, xpool, ropetmp, load_w=False):
    nc = tc.nc
    MC = M // 128
    NE = S // SLAB
    with (
        tc.tile_pool(name=f"psqk{b}", bufs=1, space="PSUM") as psqk,
        tc.tile_pool(name=f"psv{b}", bufs=2, space="PSUM") as psvp,
    ):
        xT_r = xT[b].rearrange("(mo p) s -> p mo s", p=128)
        if load_w:
            # warm the PE (HAM un-throttle needs ~3.4us of activity) with
            # junk matmuls on not-yet-loaded SBUF while the first DMAs run;
            # the results land in a psv buffer that is overwritten later
            warm = psvp.tile([128, SLAB // 128, NH * D], F32, tag="pv",
                             name="pv")
            for i in range(8):
                nc.tensor.matmul(
                    warm[:, 2 * (i % 2):2 * (i % 2) + 2, :],
                    kT_sb[:, 0, 0:128],
                    qT_sb[:, 0, 0:2 * NH * D],
                    start=True,
                    stop=True,
                )
        for e in range(NE):
            sl = slice(e * SLAB, (e + 1) * SLAB)
            xe = xpool.tile([128, MC, SLAB], BF16, tag="xe", name="xe")
            if load_w and e == 0:
                # first slab: the issue cost of dma_start on one DGE queue
                # (~0.7us each) is the startup bottleneck, so fan the
                # critical first loads out across four engines' queues
                wq_r = io["wqT"].rearrange("(mo p) n -> p mo n", p=128)
                nc.sync.dma_start(
                    xe[:, 0, :], xT[b][0:128, sl]
                )
                nc.scalar.dma_start(
                    w_sb["wqT"][:, 0, :], io["wqT"][0:128, :]
                )
                nc.gpsimd.dma_start(
                    xe[:, 1, :], xT[b][128:256, sl]
                )
                nc.gpsimd.dma_start(
                    w_sb["wqT"][:, 1, :], io["wqT"][128:256, :]
                )
                for g in range(2, MC, 4):
                    ge = min(g + 4, MC)
                    nc.sync.dma_start(xe[:, g:ge, :], xT_r[:, g:ge, sl])
                    nc.scalar.dma_start(
                        w_sb["wqT"][:, g:ge, :], wq_r[:, g:ge, :]
                    )
                nc.scalar.dma_start(
                    w_sb["wkT"][:],
                    io["wkT"].rearrange("(mo p) n -> p mo n", p=128),
                )
                nc.gpsimd.dma_start(
                    w_sb["wvT"][:],
                    io["wvT"].rearrange("(mo p) n -> p mo n", p=128),
                )
                nc.sync.dma_start(packC_sb[:, sl], io["packC"][:, sl])
                nc.sync.dma_start(packS_sb[:, sl], io["packS"][:, sl])
            else:
                nc.sync.dma_start(xe[:], xT_r[:, :, sl])
                if load_w:
                    nc.sync.dma_start(packC_sb[:, sl], io["packC"][:, sl])
                    nc.sync.dma_start(packS_sb[:, sl], io["packS"][:, sl])

            # q and k interleaved per m-chunk (4 live PSUM accumulators) so
            # each x chunk is consumed twice before the next is needed —
            # halves the DMA feed rate the PE requires on the first slab
            ps = {
                (name, h): psqk.tile([128, SLAB], F32, tag=f"p{name}{h}",
                                     name=f"p{name}{h}")
                for name in ("wqT", "wkT")
                for h in range(NH)
            }
            for m in range(MC):
                for name in ("wqT", "wkT"):
                    for h in range(NH):
                        nc.tensor.matmul(
                            ps[name, h][:],
                            w_sb[name][:, m, h * D:(h + 1) * D],
                            xe[:, m, :],
                            start=(m == 0),
                            stop=(m == MC - 1),
                        )
            for name, dst in (("wqT", qT_sb), ("wkT", kT_sb)):
                for h in range(NH):
                    # quadrant-16 RoPE: out = ps*packC + shuffle16(ps*packS)
                    t1 = ropetmp.tile([128, SLAB], F32, tag="t1", name="t1")
                    t2 = ropetmp.tile([128, SLAB], F32, tag="t2", name="t2")
                    t2s = ropetmp.tile([128, SLAB], F32, tag="t2s",
                                       name="t2s")
                    nc.vector.tensor_mul(t1[:], ps[name, h][:],
                                         packC_sb[:, sl])
                    nc.vector.tensor_mul(t2[:], ps[name, h][:],
                                         packS_sb[:, sl])
                    nc.vector.stream_shuffle(t2s[:], t2[:], SWAP16)
                    nc.vector.tensor_add(dst[:, h, sl], t1[:], t2s[:])

            # v in [seq, head_dim] layout directly: x^T chunks are the
            # stationary operand, Wv^T the moving one — no transposes
            pv = psvp.tile([128, SLAB // 128, NH * D], F32, tag="pv",
                           name="pv")
            for st in range(SLAB // 128):
                for m in range(MC):
                    nc.tensor.matmul(
                        pv[:, st, :],
                        xe[:, m, st * 128:(st + 1) * 128],
                        w_sb["wvT"][:, m, :],
                        start=(m == 0),
                        stop=(m == MC - 1),
                    )
            for st in range(SLAB // 128):
                nc.vector.tensor_copy(
                    v_sb[:, e * (SLAB // 128) + st, :], pv[:, st, :]
                )


def phase_b(tc, io, b, outT, ones_sb, tri_sb, qT_sb, kT_sb, v_sb, S):
    nc = tc.nc
    NQT = S // QT
    scale = 1.0 / math.sqrt(D)
    if b == 0:
        nc.sync.dma_start(tri_sb[:], io["tri"][:])
        nc.sync.dma_start(ones_sb[:], io["ones"][:])
    with (
        tc.tile_pool(name=f"expp{b}", bufs=4) as expp,
        tc.tile_pool(name=f"outp{b}", bufs=2) as outp,
        tc.tile_pool(name=f"psS{b}", bufs=2, space="PSUM") as psS,
        tc.tile_pool(name=f"psO{b}", bufs=2, space="PSUM") as psO,
        tc.tile_pool(name=f"psD{b}", bufs=2, space="PSUM") as psDen,
    ):
        pend = []

        def tail(expS, islot, kt, rs, nkt, out_ps, den_ps, h, u, qt):
            # den/out accumulation for one key tile; on the last key tile
            # of a query tile, also emit that tile's normalized eviction
            nc.tensor.matmul(
                den_ps[:, rs:],
                ones_sb[:],
                expS[:, islot, rs:],
                start=(kt == 0),
                stop=(kt == nkt - 1),
            )
            nc.tensor.matmul(
                out_ps[:, rs:],
                v_sb[:, kt, h * D:(h + 1) * D],
                expS[:, islot, rs:],
                start=(kt == 0),
                stop=(kt == nkt - 1),
            )
            if kt == nkt - 1:
                recip = outp.tile([128, QT], F32, tag="recip",
                                  name="recip")
                nc.vector.reciprocal_approx_fast(recip[:], den_ps[:])
                o_sb = outp.tile([128, QT], F32, tag="o", name="o_sb")
                nc.vector.tensor_mul(o_sb[:], out_ps[:], recip[:])
                nc.sync.dma_start(
                    outT[u, :, qt * QT:(qt + 1) * QT], o_sb[:]
                )

        for qt in range(NQT - 1, -1, -1):
            for h in range(NH):
                u = b * NH + h
                nkt = (qt + 1) * (QT // 128)
                out_ps = psO.tile([128, QT], F32, tag="out", name="out_ps")
                den_ps = psDen.tile([128, QT], F32, tag="den",
                                    name="den_ps")

                for p in range(nkt // 2):
                    kts = (2 * p, 2 * p + 1)
                    js = [kt - (nkt - 4) for kt in kts]
                    rss = [max(0, 128 * j) for j in js]
                    s_ps = psS.tile([128, 2, QT], F32, tag="s", name="s_ps")
                    for i in range(2):
                        nc.tensor.matmul(
                            s_ps[:, i, rss[i]:],
                            kT_sb[:, h, kts[i] * 128:(kts[i] + 1) * 128],
                            qT_sb[:, h, qt * QT + rss[i]:(qt + 1) * QT],
                            start=True,
                            stop=True,
                        )
                    expS = expp.tile([128, 2, QT], BF16, tag="exp",
                                     name="expS")
                    # one exp over both tiles; the pair's common range is
                    # the first tile's causal range (the second tile's
                    # extra columns are never read downstream)
                    crs = rss[0]
                    nc.scalar.activation(
                        expS[:, :, crs:], s_ps[:, :, crs:], AF.Exp,
                        scale=scale,
                    )
                    for i in range(2):
                        if js[i] >= 0:
                            dsl = slice(128 * js[i], 128 * (js[i] + 1))
                            nc.vector.tensor_mul(
                                expS[:, i, dsl], expS[:, i, dsl], tri_sb[:]
                            )
                        pend.append((expS, i, kts[i], rss[i], nkt,
                                     out_ps, den_ps, h, u, qt))
                    while len(pend) > 4:
                        tail(*pend.pop(0))
        while pend:
            tail(*pend.pop(0))


_NC_CACHE = {}


def _get_nc():
    if "nc" not in _NC_CACHE:
        nc = bacc.Bacc(
            "TRN2", target_bir_lowering=False, debug=False, num_devices=NCORES
        )
        io = {}
        for name, shape, dt_ in (
            ("xT0", [M, S], BF16),
            ("xT1", [M, S], BF16),
            ("wqT", [M, NH * D], BF16),
            ("wkT", [M, NH * D], BF16),
            ("wvT", [M, NH * D], BF16),
            ("packC", [128, S], F32),
            ("packS", [128, S], F32),
            ("tri", [128, 128], BF16),
            ("ones", [128, 128], BF16),
        ):
            io[name] = nc.dram_tensor(name, shape, dt_, kind="ExternalInput").ap()
        io["outT"] = nc.dram_tensor(
            "outT", [NB * NH, 128, S], F32, kind="ExternalOutput"
        ).ap()
        with tile.TileContext(nc) as tc:
            build_attention(tc, io, S, M)
        nc.compile()
        _NC_CACHE["nc"] = nc
    return _NC_CACHE["nc"]


def kernel(x, Wq, Wk, Wv):
    x = np.asarray(x, dtype=np.float32)
    Wq = np.asarray(Wq, dtype=np.float32)
    Wk = np.asarray(Wk, dtype=np.float32)
    Wv = np.asarray(Wv, dtype=np.float32)

    nc = _get_nc()
    in_maps = [prep_core_inputs(x, Wq, Wk, Wv, c, S, M) for c in range(NCORES)]
    res = bass_utils.run_bass_kernel_spmd(nc, in_maps, core_ids=list(range(NCORES)))

    out = np.empty((NB, S, M), dtype=np.float32)
    for c in range(NCORES):
        outT = res.results[c]["outT"]
        for u in range(NB * NH):
            b, hl = u // NH, u % NH
            col = c * NH * D + hl * D
            out[b, :, col:col + D] = outT[u].T
    return out


# revision 15
# speedup vs baseline: 1.0268x; 1.0268x over previous
"""Trainium2 Bass kernel for causal multi-head attention with RoPE.

nn_CausalAttention: x [2, 2048, 2048], Wq/Wk/Wv [2048, 2048] (y = x @ W.T),
16 heads of dim 128, RoPE, causal fp32 softmax.

Sharding (tensor-parallel heads, per the problem hint): each of the 8
NeuronCores owns 2 heads (a 256-wide slice of the QKV output dim) for both
batch elements. Each core runs the full pipeline for its heads; the full
output is assembled on host by concatenating per-core feature slices (no
collectives needed).

v3 (bf16 operand pipeline):
  All matmul operands are bf16 (PSUM accumulation stays fp32), which turns
  on fast-weight-load, halves HBM traffic, and removes the fp32r
  small-moving-dim penalty so diagonal attention tiles use exact causal
  widths.  Phase A computes q^T/k^T in [head_dim x seq] layout with RoPE
  fused into the PSUM eviction (quadrant-16 layout + stream_shuffle), and v
  directly in [seq x head_dim] layout by using x^T chunks as the stationary
  operand (no PE transposes at all).  Phase B runs causal attention in
  transposed-score layout with exp evaluated two key-tiles per ACTIVATE
  (PSUM quads) to amortize scalar-engine instruction overhead, softmax
  denominator via an all-ones matmul, fast approximate reciprocal fused
  into the output eviction, query tiles processed in descending order so
  the kernel tail drains the smallest tile.
"""

import math

import numpy as np
import ml_dtypes

import concourse.bacc as bacc
import concourse.bass as bass
import concourse.mybir as mybir
import concourse.tile as tile
from concourse import bass_utils

F32 = mybir.dt.float32
BF16 = mybir.dt.bfloat16
AF = mybir.ActivationFunctionType
NPBF16 = ml_dtypes.bfloat16

S = 2048
M = 2048
NCORES = 8

D = 128          # head dim
NH = 2           # heads per core
NB = 2           # batches
SLAB = 512       # phase-A sequence slab
QT = 512         # phase-B query tile


def _rope_perm(n):
    """Row permutation for the quadrant-16 RoPE layout.

    New row p (within a 128-row head block): quadrant qd = p//32, r = p%32.
    r < 16  -> even element of pair i = 16*qd + r      (old row 2i)
    r >= 16 -> odd  element of pair i = 16*qd + (r-16) (old row 2i+1)
    Pair elements are 16 partitions apart inside one 32-partition quadrant,
    so the RoPE combine is a stream_shuffle with a 16-rotation mask.
    """
    perm = []
    for hb in range(n // D):
        base = hb * D
        for qd in range(4):
            perm += [base + 2 * (16 * qd + r) for r in range(16)]
            perm += [base + 2 * (16 * qd + r) + 1 for r in range(16)]
    return np.array(perm)


SWAP16 = [(i + 16) % 32 for i in range(32)]

_PREP_CACHE = {}


def _shared_prep(x):
    key = id(x)
    if _PREP_CACHE.get("key") != key:
        _PREP_CACHE.clear()
        _PREP_CACHE["key"] = key
        _PREP_CACHE["xT0"] = np.ascontiguousarray(x[0].T).astype(NPBF16)
        _PREP_CACHE["xT1"] = np.ascontiguousarray(x[1].T).astype(NPBF16)
    return _PREP_CACHE["xT0"], _PREP_CACHE["xT1"]


def prep_core_inputs(x, Wq, Wk, Wv, core, S, M):
    """Host-side shard prep for one core. x [2,S,M], W* [M', M] where
    rows [core*256, core*256+256) of W* are this core's heads."""
    nsl = slice(core * NH * D, (core + 1) * NH * D)
    perm = _rope_perm(NH * D)
    wq = Wq[nsl][perm]
    wk = Wk[nsl][perm]
    wv = Wv[nsl]

    theta = np.exp(
        -np.float32(np.log(10000.0))
        * (np.arange(0, D, 2, dtype=np.float32) / np.float32(D))
    ).astype(np.float32)
    pos = np.arange(S, dtype=np.float32)
    freqs = theta[:, None] * pos[None, :]  # [64, S], row i = theta_i * s
    cos_t, sin_t = np.cos(freqs), np.sin(freqs)
    # quadrant-16 layout: partition p -> pair i(p) = 16*(p//32) + (p%16)
    p = np.arange(128)
    i_of_p = 16 * (p // 32) + (p % 16)
    is_odd = (p % 32) >= 16
    packC = cos_t[i_of_p].astype(np.float32)                    # [128, S]
    packS = np.where(
        is_odd[:, None], -sin_t[i_of_p], sin_t[i_of_p]
    ).astype(np.float32)

    kk, qq = np.meshgrid(np.arange(128), np.arange(128), indexing="ij")
    tri = (kk <= qq).astype(NPBF16)

    xT0, xT1 = _shared_prep(x)
    return {
        "xT0": xT0,
        "xT1": xT1,
        "wqT": np.ascontiguousarray(wq.T).astype(NPBF16),
        "wkT": np.ascontiguousarray(wk.T).astype(NPBF16),
        "wvT": np.ascontiguousarray(wv.T).astype(NPBF16),
        "packC": packC,
        "packS": packS,
        "tri": tri,
        "ones": np.ones((128, 128), dtype=NPBF16),
    }


def build_attention(tc: tile.TileContext, io: dict, S: int, M: int):
    nc = tc.nc
    xT = [io["xT0"], io["xT1"]]
    outT = io["outT"]

    with (
        tc.tile_pool(name="wpool", bufs=1) as wpool,
        tc.tile_pool(name="constpool", bufs=1) as constpool,
        tc.tile_pool(name="xp", bufs=2) as xpool,
        tc.tile_pool(name="rope", bufs=1) as ropetmp,
    ):
        MC = M // 128
        w_sb = {
            name: wpool.tile([128, MC, NH * D], BF16, tag=name, name=name)
            for name in ("wqT", "wkT", "wvT")
        }
        tri_sb = constpool.tile([128, 128], BF16)
        ones_sb = constpool.tile([128, 128], BF16)
        packC_sb = constpool.tile([128, S], F32)
        packS_sb = constpool.tile([128, S], F32)
        # (DMAs for tri/ones are issued inside phase B, packC/S inside
        #  phase A after the first slab, so they don't delay the critical
        #  first x/w loads)

        for b in range(NB):
            with tc.tile_pool(name=f"qkv{b}", bufs=1) as qkvp:
                qT_sb = qkvp.tile([128, NH, S], BF16, name="qT_sb")
                kT_sb = qkvp.tile([128, NH, S], BF16, name="kT_sb")
                v_sb = qkvp.tile([128, S // 128, NH * D], BF16, name="v_sb")

                phase_a(tc, io, b, xT, w_sb, qT_sb, kT_sb, v_sb,
                        packC_sb, packS_sb, S, M, xpool, ropetmp,
                        load_w=(b == 0))
                phase_b(tc, io, b, outT, ones_sb, tri_sb, qT_sb, kT_sb,
                        v_sb, S)


def phase_a(tc, io, b, xT, w_sb, qT_sb, kT_sb, v_sb, packC_sb, packS_sb,
            S, M, xpool, ropetmp, load_w=False):
    nc = tc.nc
    MC = M // 128
    NE = S // SLAB
    with (
        tc.tile_pool(name=f"psqk{b}", bufs=2, space="PSUM") as psqk,
        tc.tile_pool(name=f"psv{b}", bufs=2, space="PSUM") as psvp,
    ):
        xT_r = xT[b].rearrange("(mo p) s -> p mo s", p=128)
        for e in range(NE):
            sl = slice(e * SLAB, (e + 1) * SLAB)
            xe = xpool.tile([128, MC, SLAB], BF16, tag="xe", name="xe")
            if load_w and e == 0:
                # first slab: chunk the first few m so the m=0 matmuls
                # start immediately; bulk-load the rest (cheap issue)
                wq_r = io["wqT"].rearrange("(mo p) n -> p mo n", p=128)
                for m in range(4):
                    nc.sync.dma_start(
                        xe[:, m, :], xT[b][m * 128:(m + 1) * 128, sl]
                    )
                    nc.sync.dma_start(
                        w_sb["wqT"][:, m, :],
                        io["wqT"][m * 128:(m + 1) * 128, :],
                    )
                for g in range(4, MC, 4):
                    nc.sync.dma_start(
                        xe[:, g:g + 4, :], xT_r[:, g:g + 4, sl]
                    )
                    nc.sync.dma_start(
                        w_sb["wqT"][:, g:g + 4, :], wq_r[:, g:g + 4, :]
                    )
                nc.scalar.dma_start(
                    w_sb["wkT"][:],
                    io["wkT"].rearrange("(mo p) n -> p mo n", p=128),
                )
                nc.gpsimd.dma_start(
                    w_sb["wvT"][:],
                    io["wvT"].rearrange("(mo p) n -> p mo n", p=128),
                )
                nc.sync.dma_start(packC_sb[:, sl], io["packC"][:, sl])
                nc.sync.dma_start(packS_sb[:, sl], io["packS"][:, sl])
            else:
                nc.sync.dma_start(xe[:], xT_r[:, :, sl])
                if load_w:
                    nc.sync.dma_start(packC_sb[:, sl], io["packC"][:, sl])
                    nc.sync.dma_start(packS_sb[:, sl], io["packS"][:, sl])

            # q and k interleaved per m-chunk (4 live PSUM accumulators) so
            # each x chunk is consumed twice before the next is needed —
            # halves the DMA feed rate the PE requires on the first slab
            ps = {
                (name, h): psqk.tile([128, SLAB], F32, tag=f"pqk{h}",
                                     name=f"p{name}{h}")
                for name in ("wqT", "wkT")
                for h in range(NH)
            }
            for m in range(MC):
                for name in ("wqT", "wkT"):
                    for h in range(NH):
                        nc.tensor.matmul(
                            ps[name, h][:],
                            w_sb[name][:, m, h * D:(h + 1) * D],
                            xe[:, m, :],
                            start=(m == 0),
                            stop=(m == MC - 1),
                        )
            for name, dst in (("wqT", qT_sb), ("wkT", kT_sb)):
                for h in range(NH):
                    # quadrant-16 RoPE: out = ps*packC + shuffle16(ps*packS)
                    t1 = ropetmp.tile([128, SLAB], F32, tag="t1", name="t1")
                    t2 = ropetmp.tile([128, SLAB], F32, tag="t2", name="t2")
                    t2s = ropetmp.tile([128, SLAB], F32, tag="t2s",
                                       name="t2s")
                    nc.vector.tensor_mul(t1[:], ps[name, h][:],
                                         packC_sb[:, sl])
                    nc.vector.tensor_mul(t2[:], ps[name, h][:],
                                         packS_sb[:, sl])
                    nc.vector.stream_shuffle(t2s[:], t2[:], SWAP16)
                    nc.vector.tensor_add(dst[:, h, sl], t1[:], t2s[:])

            # v in [seq, head_dim] layout directly: x^T chunks are the
            # stationary operand, Wv^T the moving one — no transposes
            pv = psvp.tile([128, SLAB // 128, NH * D], F32, tag="pv",
                           name="pv")
            for st in range(SLAB // 128):
                for m in range(MC):
                    nc.tensor.matmul(
                        pv[:, st, :],
                        xe[:, m, st * 128:(st + 1) * 128],
                        w_sb["wvT"][:, m, :],
                        start=(m == 0),
                        stop=(m == MC - 1),
                    )
            for st in range(SLAB // 128):
                nc.vector.tensor_copy(
                    v_sb[:, e * (SLAB // 128) + st, :], pv[:, st, :]
                )


def phase_b(tc, io, b, outT, ones_sb, tri_sb, qT_sb, kT_sb, v_sb, S):
    nc = tc.nc
    NQT = S // QT
    scale = 1.0 / math.sqrt(D)
    if b == 0:
        nc.sync.dma_start(tri_sb[:], io["tri"][:])
        nc.sync.dma_start(ones_sb[:], io["ones"][:])
    with (
        tc.tile_pool(name=f"expp{b}", bufs=4) as expp,
        tc.tile_pool(name=f"outp{b}", bufs=2) as outp,
        tc.tile_pool(name=f"psS{b}", bufs=2, space="PSUM") as psS,
        tc.tile_pool(name=f"psO{b}", bufs=2, space="PSUM") as psO,
        tc.tile_pool(name=f"psD{b}", bufs=2, space="PSUM") as psDen,
    ):
        pend = []

        def tail(expS, islot, kt, rs, nkt, out_ps, den_ps, h, u, qt):
            # den/out accumulation for one key tile; on the last key tile
            # of a query tile, also emit that tile's normalized eviction
            nc.tensor.matmul(
                den_ps[:, rs:],
                ones_sb[:],
                expS[:, islot, rs:],
                start=(kt == 0),
                stop=(kt == nkt - 1),
            )
            nc.tensor.matmul(
                out_ps[:, rs:],
                v_sb[:, kt, h * D:(h + 1) * D],
                expS[:, islot, rs:],
                start=(kt == 0),
                stop=(kt == nkt - 1),
            )
            if kt == nkt - 1:
                recip = outp.tile([128, QT], F32, tag="recip",
                                  name="recip")
                nc.vector.reciprocal_approx_fast(recip[:], den_ps[:])
                o_sb = outp.tile([128, QT], F32, tag="o", name="o_sb")
                nc.vector.tensor_mul(o_sb[:], out_ps[:], recip[:])
                nc.sync.dma_start(
                    outT[u, :, qt * QT:(qt + 1) * QT], o_sb[:]
                )

        for qt in range(NQT - 1, -1, -1):
            for h in range(NH):
                u = b * NH + h
                nkt = (qt + 1) * (QT // 128)
                out_ps = psO.tile([128, QT], F32, tag="out", name="out_ps")
                den_ps = psDen.tile([128, QT], F32, tag="den",
                                    name="den_ps")

                for p in range(nkt // 2):
                    kts = (2 * p, 2 * p + 1)
                    js = [kt - (nkt - 4) for kt in kts]
                    rss = [max(0, 128 * j) for j in js]
                    s_ps = psS.tile([128, 2, QT], F32, tag="s", name="s_ps")
                    for i in range(2):
                        nc.tensor.matmul(
                            s_ps[:, i, rss[i]:],
                            kT_sb[:, h, kts[i] * 128:(kts[i] + 1) * 128],
                            qT_sb[:, h, qt * QT + rss[i]:(qt + 1) * QT],
                            start=True,
                            stop=True,
                        )
                    expS = expp.tile([128, 2, QT], BF16, tag="exp",
                                     name="expS")
                    # one exp over both tiles; the pair's common range is
                    # the first tile's causal range (the second tile's
                    # extra columns are never read downstream)
                    crs = rss[0]
                    nc.scalar.activation(
                        expS[:, :, crs:], s_ps[:, :, crs:], AF.Exp,
                        scale=scale,
                    )
                    for i in range(2):
                        if js[i] >= 0:
                            dsl = slice(128 * js[i], 128 * (js[i] + 1))
                            nc.vector.tensor_mul(
                                expS[:, i, dsl], expS[:, i, dsl], tri_sb[:]
                            )
                        pend.append((expS, i, kts[i], rss[i], nkt,
                                     out_ps, den_ps, h, u, qt))
                    while len(pend) > 4:
                        tail(*pend.pop(0))
        while pend:
            tail(*pend.pop(0))


_NC_CACHE = {}


def _get_nc():
    if "nc" not in _NC_CACHE:
        nc = bacc.Bacc(
            "TRN2", target_bir_lowering=False, debug=False, num_devices=NCORES
        )
        io = {}
        for name, shape, dt_ in (
            ("xT0", [M, S], BF16),
            ("xT1", [M, S], BF16),
            ("wqT", [M, NH * D], BF16),
            ("wkT", [M, NH * D], BF16),
            ("wvT", [M, NH * D], BF16),
            ("packC", [128, S], F32),
            ("packS", [128, S], F32),
            ("tri", [128, 128], BF16),
            ("ones", [128, 128], BF16),
        ):
            io[name] = nc.dram_tensor(name, shape, dt_, kind="ExternalInput").ap()
        io["outT"] = nc.dram_tensor(
            "outT", [NB * NH, 128, S], F32, kind="ExternalOutput"
        ).ap()
        with tile.TileContext(nc) as tc:
            build_attention(tc, io, S, M)
        nc.compile()
        _NC_CACHE["nc"] = nc
    return _NC_CACHE["nc"]


def kernel(x, Wq, Wk, Wv):
    x = np.asarray(x, dtype=np.float32)
    Wq = np.asarray(Wq, dtype=np.float32)
    Wk = np.asarray(Wk, dtype=np.float32)
    Wv = np.asarray(Wv, dtype=np.float32)

    nc = _get_nc()
    in_maps = [prep_core_inputs(x, Wq, Wk, Wv, c, S, M) for c in range(NCORES)]
    res = bass_utils.run_bass_kernel_spmd(nc, in_maps, core_ids=list(range(NCORES)))

    out = np.empty((NB, S, M), dtype=np.float32)
    for c in range(NCORES):
        outT = res.results[c]["outT"]
        for u in range(NB * NH):
            b, hl = u // NH, u % NH
            col = c * NH * D + hl * D
            out[b, :, col:col + D] = outT[u].T
    return out


# revision 16
# speedup vs baseline: 1.0669x; 1.0391x over previous
"""Trainium2 Bass kernel for causal multi-head attention with RoPE.

nn_CausalAttention: x [2, 2048, 2048], Wq/Wk/Wv [2048, 2048] (y = x @ W.T),
16 heads of dim 128, RoPE, causal fp32 softmax.

Sharding (tensor-parallel heads, per the problem hint): each of the 8
NeuronCores owns 2 heads (a 256-wide slice of the QKV output dim) for both
batch elements. Each core runs the full pipeline for its heads; the full
output is assembled on host by concatenating per-core feature slices (no
collectives needed).

v3 (bf16 operand pipeline):
  All matmul operands are bf16 (PSUM accumulation stays fp32), which turns
  on fast-weight-load, halves HBM traffic, and removes the fp32r
  small-moving-dim penalty so diagonal attention tiles use exact causal
  widths.  Phase A computes q^T/k^T in [head_dim x seq] layout with RoPE
  fused into the PSUM eviction (quadrant-16 layout + stream_shuffle), and v
  directly in [seq x head_dim] layout by using x^T chunks as the stationary
  operand (no PE transposes at all).  Phase B runs causal attention in
  transposed-score layout with exp evaluated two key-tiles per ACTIVATE
  (PSUM quads) to amortize scalar-engine instruction overhead, softmax
  denominator via an all-ones matmul, fast approximate reciprocal fused
  into the output eviction, query tiles processed in descending order so
  the kernel tail drains the smallest tile.
"""

import math

import numpy as np
import ml_dtypes

import concourse.bacc as bacc
import concourse.bass as bass
import concourse.mybir as mybir
import concourse.tile as tile
from concourse import bass_utils

F32 = mybir.dt.float32
BF16 = mybir.dt.bfloat16
AF = mybir.ActivationFunctionType
NPBF16 = ml_dtypes.bfloat16

S = 2048
M = 2048
NCORES = 8

D = 128          # head dim
NH = 2           # heads per core
NB = 2           # batches
SLAB = 512       # phase-A sequence slab
QT = 512         # phase-B query tile


def _rope_perm(n):
    """Row permutation for the quadrant-16 RoPE layout.

    New row p (within a 128-row head block): quadrant qd = p//32, r = p%32.
    r < 16  -> even element of pair i = 16*qd + r      (old row 2i)
    r >= 16 -> odd  element of pair i = 16*qd + (r-16) (old row 2i+1)
    Pair elements are 16 partitions apart inside one 32-partition quadrant,
    so the RoPE combine is a stream_shuffle with a 16-rotation mask.
    """
    perm = []
    for hb in range(n // D):
        base = hb * D
        for qd in range(4):
            perm += [base + 2 * (16 * qd + r) for r in range(16)]
            perm += [base + 2 * (16 * qd + r) + 1 for r in range(16)]
    return np.array(perm)


SWAP16 = [(i + 16) % 32 for i in range(32)]

_PREP_CACHE = {}


def _shared_prep(x):
    key = id(x)
    if _PREP_CACHE.get("key") != key:
        _PREP_CACHE.clear()
        _PREP_CACHE["key"] = key
        _PREP_CACHE["xT0"] = np.ascontiguousarray(x[0].T).astype(NPBF16)
        _PREP_CACHE["xT1"] = np.ascontiguousarray(x[1].T).astype(NPBF16)
    return _PREP_CACHE["xT0"], _PREP_CACHE["xT1"]


def prep_core_inputs(x, Wq, Wk, Wv, core, S, M):
    """Host-side shard prep for one core. x [2,S,M], W* [M', M] where
    rows [core*256, core*256+256) of W* are this core's heads."""
    nsl = slice(core * NH * D, (core + 1) * NH * D)
    perm = _rope_perm(NH * D)
    wq = Wq[nsl][perm]
    wk = Wk[nsl][perm]
    wv = Wv[nsl]

    theta = np.exp(
        -np.float32(np.log(10000.0))
        * (np.arange(0, D, 2, dtype=np.float32) / np.float32(D))
    ).astype(np.float32)
    pos = np.arange(S, dtype=np.float32)
    freqs = theta[:, None] * pos[None, :]  # [64, S], row i = theta_i * s
    cos_t, sin_t = np.cos(freqs), np.sin(freqs)
    # quadrant-16 layout: partition p -> pair i(p) = 16*(p//32) + (p%16)
    p = np.arange(128)
    i_of_p = 16 * (p // 32) + (p % 16)
    is_odd = (p % 32) >= 16
    packC = cos_t[i_of_p].astype(np.float32)                    # [128, S]
    packS = np.where(
        is_odd[:, None], -sin_t[i_of_p], sin_t[i_of_p]
    ).astype(np.float32)

    kk, qq = np.meshgrid(np.arange(128), np.arange(128), indexing="ij")
    tri = (kk <= qq).astype(NPBF16)

    xT0, xT1 = _shared_prep(x)
    return {
        "xT0": xT0,
        "xT1": xT1,
        "wqT": np.ascontiguousarray(wq.T).astype(NPBF16),
        "wkT": np.ascontiguousarray(wk.T).astype(NPBF16),
        "wvT": np.ascontiguousarray(wv.T).astype(NPBF16),
        "packC": packC,
        "packS": packS,
        "tri": tri,
        "ones": np.ones((128, 128), dtype=NPBF16),
    }


def build_attention(tc: tile.TileContext, io: dict, S: int, M: int):
    nc = tc.nc
    xT = [io["xT0"], io["xT1"]]
    outT = io["outT"]

    with (
        tc.tile_pool(name="wpool", bufs=1) as wpool,
        tc.tile_pool(name="constpool", bufs=1) as constpool,
        tc.tile_pool(name="xp", bufs=2) as xpool,
        tc.tile_pool(name="rope", bufs=1) as ropetmp,
    ):
        MC = M // 128
        w_sb = {
            name: wpool.tile([128, MC, NH * D], BF16, tag=name, name=name)
            for name in ("wqT", "wkT", "wvT")
        }
        tri_sb = constpool.tile([128, 128], BF16)
        ones_sb = constpool.tile([128, 128], BF16)
        packC_sb = constpool.tile([128, S], F32)
        packS_sb = constpool.tile([128, S], F32)
        # (DMAs for tri/ones are issued inside phase B, packC/S inside
        #  phase A after the first slab, so they don't delay the critical
        #  first x/w loads)

        for b in range(NB):
            with tc.tile_pool(name=f"qkv{b}", bufs=1) as qkvp:
                qT_sb = qkvp.tile([128, NH, S], BF16, name="qT_sb")
                kT_sb = qkvp.tile([128, NH, S], BF16, name="kT_sb")
                v_sb = qkvp.tile([128, S // 128, NH * D], BF16, name="v_sb")

                phase_a(tc, io, b, xT, w_sb, qT_sb, kT_sb, v_sb,
                        packC_sb, packS_sb, S, M, xpool, ropetmp,
                        load_w=(b == 0))
                phase_b(tc, io, b, outT, ones_sb, tri_sb, qT_sb, kT_sb,
                        v_sb, S)


def phase_a(tc, io, b, xT, w_sb, qT_sb, kT_sb, v_sb, packC_sb, packS_sb,
            S, M, xpool, ropetmp, load_w=False):
    nc = tc.nc
    MC = M // 128
    NE = S // SLAB
    with (
        tc.tile_pool(name=f"psqk{b}", bufs=2, space="PSUM") as psqk,
        tc.tile_pool(name=f"psv{b}", bufs=2, space="PSUM") as psvp,
    ):
        xT_r = xT[b].rearrange("(mo p) s -> p mo s", p=128)
        for e in range(NE):
            sl = slice(e * SLAB, (e + 1) * SLAB)
            xe = xpool.tile([128, MC, SLAB], BF16, tag="xe", name="xe")
            if load_w and e == 0:
                # first slab: chunk the first few m so the m=0 matmuls
                # start immediately; bulk-load the rest (cheap issue)
                wq_r = io["wqT"].rearrange("(mo p) n -> p mo n", p=128)
                for m in range(4):
                    nc.sync.dma_start(
                        xe[:, m, :], xT[b][m * 128:(m + 1) * 128, sl]
                    )
                    nc.sync.dma_start(
                        w_sb["wqT"][:, m, :],
                        io["wqT"][m * 128:(m + 1) * 128, :],
                    )
                for g in range(4, MC, 4):
                    nc.sync.dma_start(
                        xe[:, g:g + 4, :], xT_r[:, g:g + 4, sl]
                    )
                    nc.sync.dma_start(
                        w_sb["wqT"][:, g:g + 4, :], wq_r[:, g:g + 4, :]
                    )
                nc.scalar.dma_start(
                    w_sb["wkT"][:],
                    io["wkT"].rearrange("(mo p) n -> p mo n", p=128),
                )
                nc.gpsimd.dma_start(
                    w_sb["wvT"][:],
                    io["wvT"].rearrange("(mo p) n -> p mo n", p=128),
                )
                nc.sync.dma_start(packC_sb[:, sl], io["packC"][:, sl])
                nc.sync.dma_start(packS_sb[:, sl], io["packS"][:, sl])
            else:
                nc.sync.dma_start(xe[:], xT_r[:, :, sl])
                if load_w:
                    nc.sync.dma_start(packC_sb[:, sl], io["packC"][:, sl])
                    nc.sync.dma_start(packS_sb[:, sl], io["packS"][:, sl])

            for name, dst in (("wqT", qT_sb), ("wkT", kT_sb)):
                ps = [
                    psqk.tile([128, SLAB], F32, tag=f"pqk{h}",
                              name=f"pqk{h}")
                    for h in range(NH)
                ]
                for m in range(MC):
                    for h in range(NH):
                        nc.tensor.matmul(
                            ps[h][:],
                            w_sb[name][:, m, h * D:(h + 1) * D],
                            xe[:, m, :],
                            start=(m == 0),
                            stop=(m == MC - 1),
                        )
                for h in range(NH):
                    # quadrant-16 RoPE: out = ps*packC + shuffle16(ps*packS)
                    t1 = ropetmp.tile([128, SLAB], F32, tag="t1", name="t1")
                    t2 = ropetmp.tile([128, SLAB], F32, tag="t2", name="t2")
                    t2s = ropetmp.tile([128, SLAB], F32, tag="t2s",
                                       name="t2s")
                    nc.vector.tensor_mul(t1[:], ps[h][:], packC_sb[:, sl])
                    nc.vector.tensor_mul(t2[:], ps[h][:], packS_sb[:, sl])
                    nc.vector.stream_shuffle(t2s[:], t2[:], SWAP16)
                    nc.vector.tensor_add(dst[:, h, sl], t1[:], t2s[:])

            # v in [seq, head_dim] layout directly: x^T chunks are the
            # stationary operand, Wv^T the moving one — no transposes
            pv = psvp.tile([128, SLAB // 128, NH * D], F32, tag="pv",
                           name="pv")
            for st in range(SLAB // 128):
                for m in range(MC):
                    nc.tensor.matmul(
                        pv[:, st, :],
                        xe[:, m, st * 128:(st + 1) * 128],
                        w_sb["wvT"][:, m, :],
                        start=(m == 0),
                        stop=(m == MC - 1),
                    )
            for st in range(SLAB // 128):
                nc.vector.tensor_copy(
                    v_sb[:, e * (SLAB // 128) + st, :], pv[:, st, :]
                )


def phase_b(tc, io, b, outT, ones_sb, tri_sb, qT_sb, kT_sb, v_sb, S):
    nc = tc.nc
    NQT = S // QT
    scale = 1.0 / math.sqrt(D)
    if b == 0:
        nc.sync.dma_start(tri_sb[:], io["tri"][:])
        nc.sync.dma_start(ones_sb[:], io["ones"][:])
    with (
        tc.tile_pool(name=f"expp{b}", bufs=4) as expp,
        tc.tile_pool(name=f"outp{b}", bufs=2) as outp,
        tc.tile_pool(name=f"psS{b}", bufs=2, space="PSUM") as psS,
        tc.tile_pool(name=f"psO{b}", bufs=2, space="PSUM") as psO,
        tc.tile_pool(name=f"psD{b}", bufs=2, space="PSUM") as psDen,
    ):
        pend = []

        def tail(expS, islot, kt, rs, nkt, out_ps, den_ps, h, u, qt):
            # den/out accumulation for one key tile; on the last key tile
            # of a query tile, also emit that tile's normalized eviction
            nc.tensor.matmul(
                den_ps[:, rs:],
                ones_sb[:],
                expS[:, islot, rs:],
                start=(kt == 0),
                stop=(kt == nkt - 1),
            )
            nc.tensor.matmul(
                out_ps[:, rs:],
                v_sb[:, kt, h * D:(h + 1) * D],
                expS[:, islot, rs:],
                start=(kt == 0),
                stop=(kt == nkt - 1),
            )
            if kt == nkt - 1:
                recip = outp.tile([128, QT], F32, tag="recip",
                                  name="recip")
                nc.vector.reciprocal_approx_fast(recip[:], den_ps[:])
                o_sb = outp.tile([128, QT], F32, tag="o", name="o_sb")
                nc.vector.tensor_mul(o_sb[:], out_ps[:], recip[:])
                nc.sync.dma_start(
                    outT[u, :, qt * QT:(qt + 1) * QT], o_sb[:]
                )

        for qt in range(NQT - 1, -1, -1):
            for h in range(NH):
                u = b * NH + h
                nkt = (qt + 1) * (QT // 128)
                out_ps = psO.tile([128, QT], F32, tag="out", name="out_ps")
                den_ps = psDen.tile([128, QT], F32, tag="den",
                                    name="den_ps")

                for p in range(nkt // 2):
                    kts = (2 * p, 2 * p + 1)
                    js = [kt - (nkt - 4) for kt in kts]
                    rss = [max(0, 128 * j) for j in js]
                    s_ps = psS.tile([128, 2, QT], F32, tag="s", name="s_ps")
                    for i in range(2):
                        nc.tensor.matmul(
                            s_ps[:, i, rss[i]:],
                            kT_sb[:, h, kts[i] * 128:(kts[i] + 1) * 128],
                            qT_sb[:, h, qt * QT + rss[i]:(qt + 1) * QT],
                            start=True,
                            stop=True,
                        )
                    expS = expp.tile([128, 2, QT], BF16, tag="exp",
                                     name="expS")
                    # one exp over both tiles; the pair's common range is
                    # the first tile's causal range (the second tile's
                    # extra columns are never read downstream)
                    crs = rss[0]
                    nc.scalar.activation(
                        expS[:, :, crs:], s_ps[:, :, crs:], AF.Exp,
                        scale=scale,
                    )
                    for i in range(2):
                        if js[i] >= 0:
                            dsl = slice(128 * js[i], 128 * (js[i] + 1))
                            nc.vector.tensor_mul(
                                expS[:, i, dsl], expS[:, i, dsl], tri_sb[:]
                            )
                        pend.append((expS, i, kts[i], rss[i], nkt,
                                     out_ps, den_ps, h, u, qt))
                    while len(pend) > 4:
                        tail(*pend.pop(0))
        while pend:
            tail(*pend.pop(0))


_NC_CACHE = {}


def _get_nc():
    if "nc" not in _NC_CACHE:
        nc = bacc.Bacc(
            "TRN2", target_bir_lowering=False, debug=False, num_devices=NCORES
        )
        io = {}
        for name, shape, dt_ in (
            ("xT0", [M, S], BF16),
            ("xT1", [M, S], BF16),
            ("wqT", [M, NH * D], BF16),
            ("wkT", [M, NH * D], BF16),
            ("wvT", [M, NH * D], BF16),
            ("packC", [128, S], F32),
            ("packS", [128, S], F32),
            ("tri", [128, 128], BF16),
            ("ones", [128, 128], BF16),
        ):
            io[name] = nc.dram_tensor(name, shape, dt_, kind="ExternalInput").ap()
        io["outT"] = nc.dram_tensor(
            "outT", [NB * NH, 128, S], F32, kind="ExternalOutput"
        ).ap()
        with tile.TileContext(nc) as tc:
            build_attention(tc, io, S, M)
        nc.compile()
        _NC_CACHE["nc"] = nc
    return _NC_CACHE["nc"]


def kernel(x, Wq, Wk, Wv):
    x = np.asarray(x, dtype=np.float32)
    Wq = np.asarray(Wq, dtype=np.float32)
    Wk = np.asarray(Wk, dtype=np.float32)
    Wv = np.asarray(Wv, dtype=np.float32)

    nc = _get_nc()
    in_maps = [prep_core_inputs(x, Wq, Wk, Wv, c, S, M) for c in range(NCORES)]
    res = bass_utils.run_bass_kernel_spmd(nc, in_maps, core_ids=list(range(NCORES)))

    out = np.empty((NB, S, M), dtype=np.float32)
    for c in range(NCORES):
        outT = res.results[c]["outT"]
        for u in range(NB * NH):
            b, hl = u // NH, u % NH
            col = c * NH * D + hl * D
            out[b, :, col:col + D] = outT[u].T
    return out
